# revision 1
# baseline (speedup 1.0000x reference)
"""AttentionalSplatting Trainium2 kernel (8 NeuronCores, SPMD).

Sharding: 8 cores = T(4) x HW-halves(2).  Core c handles t = c//2 and pixel
columns [ (c%2)*1152, (c%2+1)*1152 ).  Each core runs the full pipeline for
its (t, pixel-half): coord-proj + 2D RoPE -> Q/K/V proj -> qk-norm ->
scores(+spatial bias) -> softmax -> attend -> Wo -> W_out -> residual.
No cross-core communication is needed; outputs concatenate.

On-chip layout is feature-major ("transposed"): feature/head dims live on
SBUF partitions, pixels/tokens on the free dim.  Scores are computed as
S^T (m on partitions, q free) so the attend matmul consumes exp(S^T)
directly and softmax sums arrive free via a ones-column appended to V.
The spatial bias enters PSUM through identity matmuls so the exp can read
(scores+bias) straight out of PSUM on the scalar engine.
"""

import math
import sys

import numpy as np

sys.path.insert(0, "/opt/trn_rl_repo")

import ml_dtypes  # noqa: E402

import concourse.bass as bass  # noqa: E402
import concourse.bacc as bacc  # noqa: E402
import concourse.tile as tile  # noqa: E402
from concourse import mybir  # noqa: E402
from concourse.bass_utils import run_bass_kernel_spmd  # noqa: E402

T, M, HW, DF, H = 4, 1024, 2304, 256, 8
DKH = DF // H  # 32
QH = HW // 2  # 1152 pixels per core
SCALE = 1.0 / math.sqrt(DKH)
D_HALF = DF // 2  # 128
D_QUART = DF // 4  # 64
THETA = (100.0 ** (-2.0 * np.arange(D_QUART, dtype=np.float32) / D_HALF)).astype(
    np.float32
)

F32 = mybir.dt.float32
BF16 = mybir.dt.bfloat16
AF = mybir.ActivationFunctionType
BF16NP = ml_dtypes.bfloat16

N_CORES = 8
Q_BLOCKS = [(0, 512), (512, 512), (1024, 128)]
M_TRIPS = [(0, 1, 2), (3, 4, 5), (6, 7)]
K_CHUNKS = [(0, 512), (512, 512)]


def _bf(x):
    return np.ascontiguousarray(np.asarray(x, np.float32)).astype(BF16NP)


def _f32(x):
    return np.ascontiguousarray(np.asarray(x, np.float32))


def _host_constants(Wq, Wk, Wv, Wo, W_out_w, W_out_b, W_coord_w, W_coord_b):
    """Shared (core-independent) device constants, host-precomputed."""
    # pair-swapped coord weights for RoPE (swap even/odd output columns)
    perm = np.arange(DF)
    perm = perm.reshape(-1, 2)[:, ::-1].reshape(-1)
    wcsw = W_coord_w[:, perm]
    wcbsw = W_coord_b[perm]
    # signed duplicated theta: [-t0, +t0, -t1, +t1, ...]
    th = np.empty((1, D_HALF), np.float32)
    th[0, 0::2] = -THETA
    th[0, 1::2] = THETA
    # block-ones for per-head sum of squares: dtile k maps its 128 feature
    # rows onto head columns 4k..4k+3
    bones = np.zeros((2, 128, 8), np.float32)
    for k in range(2):
        for d in range(128):
            bones[k, d, 4 * k + d // 32] = 1.0
    # expand per-head scalars (8, q) back to the 128 feature rows of dtile k
    exp8 = np.zeros((2, 8, 128), np.float32)
    for k in range(2):
        for d in range(128):
            exp8[k, 4 * k + d // 32, d] = 1.0
    # expand per-head inv-sum (8, q) to paired attend-output rows:
    # pair j holds head 2j at rows 1..33 and head 2j+1 at rows 65..97
    expP = np.zeros((4, 8, 128), np.float32)
    for j in range(4):
        expP[j, 2 * j, 1:33] = 1.0
        expP[j, 2 * j + 1, 65:97] = 1.0
    # Wo rearranged to the paired attend-output row layout (sumexp rows = 0)
    wo_aug = np.zeros((4, 128, DF), np.float32)
    for j in range(4):
        wo_aug[j, 1:33, :] = Wo[(2 * j) * 32 : (2 * j + 1) * 32, :]
        wo_aug[j, 65:97, :] = Wo[(2 * j + 1) * 32 : (2 * j + 2) * 32, :]
    return {
        "wq": _bf(Wq),
        "wk": _bf(Wk),
        "wv": _bf(Wv),
        "wo_aug": _bf(wo_aug),
        "wout": _bf(W_out_w),
        "woutb": _f32(W_out_b).reshape(DF, 1),
        "wc": _f32(W_coord_w),
        "wcb": _f32(W_coord_b).reshape(DF, 1),
        "wcsw": _f32(wcsw),
        "wcbsw": _f32(wcbsw).reshape(DF, 1),
        "theta2s": th,
        "bones": bones,
        "exp8": exp8,
        "expP": expP,
        "ident": _bf(np.eye(128, dtype=np.float32)),
    }


_NC_CACHE = None


def _build_nc():
    nc = bacc.Bacc(
        "TRN2",
        target_bir_lowering=False,
        debug=False,
        enable_asserts=True,
        num_devices=N_CORES,
    )
    d = {}
    inp = lambda name, shape, dt: d.__setitem__(
        name, nc.declare_dram_parameter(name, list(shape), dt, isOutput=False)
    )
    inp("tokT", (DF, M), BF16)
    inp("posT", (2, QH), F32)
    inp("biasT", (M, QH), BF16)
    inp("fmapT", (DF, QH), F32)
    inp("wq", (DF, DF), BF16)
    inp("wk", (DF, DF), BF16)
    inp("wv", (DF, DF), BF16)
    inp("wo_aug", (4, 128, DF), BF16)
    inp("wout", (DF, DF), BF16)
    inp("woutb", (DF, 1), F32)
    inp("wc", (2, DF), F32)
    inp("wcb", (DF, 1), F32)
    inp("wcsw", (2, DF), F32)
    inp("wcbsw", (DF, 1), F32)
    inp("theta2s", (1, D_HALF), F32)
    inp("bones", (2, 128, 8), F32)
    inp("exp8", (2, 8, 128), F32)
    inp("expP", (4, 8, 128), F32)
    inp("ident", (128, 128), BF16)
    out = nc.declare_dram_parameter("out", [DF, QH], F32, isOutput=True)

    import os as _os

    with tile.TileContext(
        nc, trace_sim=bool(_os.environ.get("KERNEL_TRACE_SIM"))
    ) as tc:
        _body(nc, tc, d, out)
    nc.compile()
    return nc


def _body(nc, tc, d, out_dram):
    mm = nc.tensor.matmul
    act = nc.scalar.activation
    dma = nc.sync.dma_start

    with (
        tc.tile_pool(name="const", bufs=1) as cpool,
        tc.tile_pool(name="work", bufs=1) as wpool,
        tc.tile_pool(name="persist", bufs=1) as ppool,
        tc.tile_pool(name="epool", bufs=4) as epool,
        tc.tile_pool(name="psA", bufs=2, space=bass.MemorySpace.PSUM) as psA,
        tc.tile_pool(name="psB", bufs=2, space=bass.MemorySpace.PSUM) as psB,
    ):
        # ---- load constants / inputs to SBUF ----
        # 256-row tensors fold to (128, 2, ...): [:, kt, ...] = rows kt*128..
        def load(name, shape, dt, rearrange=None, **kw):
            t = cpool.tile(list(shape), dt, tag=name)
            src = d[name][:]
            if rearrange is not None:
                src = src.rearrange(rearrange, **kw)
            dma(t[:], src)
            return t

        fold = "(k p) d -> p k d"
        wq = load("wq", (128, 2, DF), BF16, fold, p=128)
        wk = load("wk", (128, 2, DF), BF16, fold, p=128)
        wv = load("wv", (128, 2, DF), BF16, fold, p=128)
        wo_aug = load("wo_aug", (128, 4, DF), BF16, "j p d -> p j d")
        wout = load("wout", (128, 2, DF), BF16, fold, p=128)
        woutb = load("woutb", (128, 2, 1), F32, fold, p=128)
        wc = load("wc", (2, DF), F32)
        wcb = load("wcb", (128, 2, 1), F32, fold, p=128)
        wcsw = load("wcsw", (2, DF), F32)
        wcbsw = load("wcbsw", (128, 2, 1), F32, fold, p=128)
        th2 = load("theta2s", (1, D_HALF), F32)
        bones = load("bones", (128, 2, 8), F32, "k p h -> p k h")
        exp8 = load("exp8", (8, 2, 128), F32, "k h d -> h k d")
        expP = load("expP", (8, 4, 128), F32, "j s e -> s j e")
        ident = load("ident", (128, 128), BF16)
        posT = load("posT", (2, QH), F32)
        posT2 = cpool.tile([1, 2, QH], F32, tag="posT2")
        dma(posT2[:], d["posT"][:].rearrange("(o a) q -> o a q", o=1))
        fmapT = load("fmapT", (128, 2, QH), F32, fold, p=128)
        tokT = load("tokT", (128, 2, M), BF16, fold, p=128)

        bias_sb = []
        for mc in range(8):
            bt = ppool.tile([128, QH], BF16, tag=f"bias{mc}")
            dma(bt[:], d["biasT"][mc * 128 : (mc + 1) * 128, :])
            bias_sb.append(bt)

        def const_tile(val, name):
            t = cpool.tile([128, 1], F32, tag=name)
            nc.vector.memset(t[:], val)
            return t

        halfpi = const_tile(math.pi / 2.0, "halfpi")
        zero_c = const_tile(0.0, "zeroc")
        lnscale = const_tile(math.log(SCALE), "lnscale")

        # ---- Q path: Qin^T (and pair-swapped) = Wc^T @ pos^T (+b) ----
        qin = []
        for dt_i, (w, b) in enumerate(((wc, wcb), (wcsw, wcbsw))):
            for half in range(2):
                ps = psA.tile([128, QH], F32, tag="big")
                for qo, qb in Q_BLOCKS:
                    mm(
                        ps[:, qo : qo + qb],
                        w[:, half * 128 : (half + 1) * 128],
                        posT[:, qo : qo + qb],
                    )
                t = wpool.tile([128, QH], BF16, tag=f"qin{dt_i}{half}")
                nc.vector.tensor_scalar_add(t[:], ps[:], b[:, half, :])
                qin.append(t)
        qin0, qin1, qins0, qins1 = qin

        # ---- RoPE tables: one angle matmul per axis (signed theta) ----
        cs = []
        for axis in range(2):
            ps = psA.tile([128, QH], F32, tag="big")
            for qo, qb in Q_BLOCKS:
                mm(
                    ps[:, qo : qo + qb],
                    th2[:, :],
                    posT2[:, axis, qo : qo + qb],
                )
            c_t = wpool.tile([128, QH], BF16, tag=f"cos{axis}")
            s_t = wpool.tile([128, QH], BF16, tag=f"sin{axis}")
            act(c_t[:], ps[:], AF.Sin, bias=halfpi[:])
            act(s_t[:], ps[:], AF.Sin)
            cs.append((c_t, s_t))

        roped = []
        for dt_i, (q, qs) in enumerate(((qin0, qins0), (qin1, qins1))):
            c_t, s_t = cs[dt_i]
            t1 = wpool.tile([128, QH], BF16, tag=f"ropea{dt_i}")
            nc.vector.tensor_mul(t1[:], q[:], c_t[:])
            t2 = wpool.tile([128, QH], BF16, tag=f"ropeb{dt_i}")
            nc.vector.tensor_mul(t2[:], qs[:], s_t[:])
            r = wpool.tile([128, QH], BF16, tag=f"roped{dt_i}")
            nc.vector.tensor_add(r[:], t1[:], t2[:])
            roped.append(r)

        # ---- Q = roped @ Wq  (computed as Q^T, feature-major) ----
        def proj_T(w_sb, rhs_tiles, n, blocks, name):
            """out^T[dt] (128, n) = sum_kt w[kt,dt]^T @ rhs[kt]; returns psum tiles"""
            outs = []
            for dt_i in range(2):
                ps = psA.tile([128, n], F32, tag="big")
                for qo, qb in blocks:
                    for kt in range(2):
                        mm(
                            ps[:, qo : qo + qb],
                            w_sb[:, kt, dt_i * 128 : (dt_i + 1) * 128],
                            rhs_tiles[kt][:, qo : qo + qb],
                            start=(kt == 0),
                            stop=(kt == 1),
                        )
                outs.append(ps)
            return outs

        def qknorm(ps_list, n, blocks, ln_bias, name):
            """psum (128, n) x2 -> normalized bf16 tiles (128, n) x2"""
            sq_ps = psA.tile([8, n], F32, tag="big")
            bf_tiles = []
            for dt_i, ps in enumerate(ps_list):
                tb = ppool.tile([128, n], BF16, tag=f"{name}n{dt_i}")
                nc.vector.tensor_copy(tb[:], ps[:])
                sq = wpool.tile([128, n], F32, tag=f"{name}f{dt_i}")
                nc.vector.tensor_copy(sq[:], ps[:])
                nc.vector.tensor_mul(sq[:], sq[:], sq[:])
                for qo, qb in blocks:
                    mm(
                        sq_ps[:, qo : qo + qb],
                        bones[:, dt_i, :],
                        sq[:, qo : qo + qb],
                        start=(dt_i == 0),
                        stop=(dt_i == 1),
                    )
                bf_tiles.append(tb)
            lnt = wpool.tile([8, n], F32, tag=f"{name}ln")
            act(lnt[:], sq_ps[:], AF.Ln)
            if ln_bias is None:
                ln_bias = zero_c
            invn = wpool.tile([8, n], F32, tag=f"{name}inv")
            act(invn[:], lnt[:], AF.Exp, scale=-0.5, bias=ln_bias[:8, :])
            outs = []
            for dt_i, tb in enumerate(bf_tiles):
                psx = psA.tile([128, n], F32, tag="big")
                for qo, qb in blocks:
                    mm(psx[:, qo : qo + qb], exp8[:, dt_i, :], invn[:, qo : qo + qb])
                tn = ppool.tile([128, n], BF16, tag=f"{name}T{dt_i}")
                nc.vector.tensor_mul(tn[:], tb[:], psx[:])
                outs.append(tn)
            return outs

        q_ps = proj_T(wq, roped, QH, Q_BLOCKS, "q")
        qnT = qknorm(q_ps, QH, Q_BLOCKS, lnscale, "q")

        tok_tiles = [tokT[:, 0, :], tokT[:, 1, :]]
        k_ps = proj_T(wk, tok_tiles, M, K_CHUNKS, "k")
        knT = qknorm(k_ps, M, K_CHUNKS, None, "k")

        # ---- V (token-major) with ones column:  vsb[mc] = (128, 8, 33) ----
        vsb = []
        for mc in range(8):
            ps = psB.tile([128, 256], F32, tag="small")
            for kt in range(2):
                mm(
                    ps[:],
                    tokT[:, kt, mc * 128 : (mc + 1) * 128],
                    wv[:, kt, :],
                    start=(kt == 0),
                    stop=(kt == 1),
                )
            vt = ppool.tile([128, 8, 33], BF16, tag=f"v{mc}")
            nc.vector.memset(vt[:, :, 0:1], 1.0)
            nc.vector.tensor_copy(
                vt[:, :, 1:33], ps[:].rearrange("p (h e) -> p h e", h=8)
            )
            vsb.append(vt)

        # ---- main attention loop ----
        # pair j: head 2j accumulates at psum rows 0..32, head 2j+1 at 64..96
        osb = []  # per pair (128, QH) bf16, rows 0/64 = sumexp
        for j in range(4):
            t = ppool.tile([128, QH], BF16, tag=f"osb{j}")
            osb.append(t)

        for qo, qb in Q_BLOCKS:
            for j in range(4):
                heads = (2 * j, 2 * j + 1)
                o_ps = psB.tile([128, qb], F32, tag="small")
                for trip in M_TRIPS:
                    w3 = len(trip) * qb
                    e_ts = {}
                    s_tiles = {}
                    for h in heads:
                        dt_i = h // 4
                        hp = (h % 4) * 32
                        s_ps = psA.tile([128, w3], F32, tag="big")
                        s_tiles[h] = s_ps
                        for i, mc in enumerate(trip):
                            mm(
                                s_ps[:, i * qb : (i + 1) * qb],
                                ident[:],
                                bias_sb[mc][:, qo : qo + qb],
                                start=True,
                                stop=False,
                            )
                    for i, mc in enumerate(trip):
                        for h in heads:
                            dt_i = h // 4
                            hp = (h % 4) * 32
                            mm(
                                s_tiles[h][:, i * qb : (i + 1) * qb],
                                knT[dt_i][hp : hp + 32, mc * 128 : (mc + 1) * 128],
                                qnT[dt_i][hp : hp + 32, qo : qo + qb],
                                start=False,
                                stop=True,
                                tile_position=(hp, 0),
                            )
                    for h in heads:
                        e_t = epool.tile([128, 3 * qb], BF16, tag="E")
                        act(e_t[:, 0:w3], s_tiles[h][:], AF.Exp)
                        e_ts[h] = e_t
                    for i, mc in enumerate(trip):
                        for h in heads:
                            base = 64 * (h % 2)
                            mm(
                                o_ps[base : base + 33, :],
                                vsb[mc][:, h, :],
                                e_ts[h][:, i * qb : (i + 1) * qb],
                                start=(mc == 0),
                                stop=(mc == 7),
                                tile_position=(0, base),
                            )
                nc.vector.tensor_copy(osb[j][:, qo : qo + qb], o_ps[:])

        # ---- softmax denominators: gather row 0 of each head, invert ----
        sumE = wpool.tile([8, QH], BF16, tag="sumE")
        for h in range(8):
            j, r = h // 2, 64 * (h % 2)
            dma(sumE[h : h + 1, :], osb[h // 2][r : r + 1, :])
        lnS = wpool.tile([8, QH], F32, tag="lnS")
        act(lnS[:], sumE[:], AF.Ln)
        invS = wpool.tile([8, QH], F32, tag="invS")
        act(invS[:], lnS[:], AF.Exp, scale=-1.0)

        for j in range(4):
            for qo, qb in Q_BLOCKS:
                ps = psB.tile([128, qb], F32, tag="small")
                mm(ps[:], expP[:, j, :], invS[:, qo : qo + qb])
                nc.vector.tensor_mul(
                    osb[j][:, qo : qo + qb], osb[j][:, qo : qo + qb], ps[:]
                )

        # ---- output projections + residual ----
        o1b = []
        for dt_i in range(2):
            ps = psA.tile([128, QH], F32, tag="big")
            for qo, qb in Q_BLOCKS:
                for j in range(4):
                    mm(
                        ps[:, qo : qo + qb],
                        wo_aug[:, j, dt_i * 128 : (dt_i + 1) * 128],
                        osb[j][:, qo : qo + qb],
                        start=(j == 0),
                        stop=(j == 3),
                    )
            t = wpool.tile([128, QH], BF16, tag=f"o1b{dt_i}")
            nc.vector.tensor_copy(t[:], ps[:])
            o1b.append(t)

        for dt_i in range(2):
            ps = psA.tile([128, QH], F32, tag="big")
            for qo, qb in Q_BLOCKS:
                for kt in range(2):
                    mm(
                        ps[:, qo : qo + qb],
                        wout[:, kt, dt_i * 128 : (dt_i + 1) * 128],
                        o1b[kt][:, qo : qo + qb],
                        start=(kt == 0),
                        stop=(kt == 1),
                    )
            r1 = wpool.tile([128, QH], F32, tag=f"res{dt_i}")
            nc.vector.tensor_scalar_add(r1[:], ps[:], woutb[:, dt_i, :])
            nc.vector.tensor_add(r1[:], r1[:], fmapT[:, dt_i, :])
            dma(out_dram[dt_i * 128 : (dt_i + 1) * 128, :], r1[:])


def kernel(
    track_tokens,
    feature_map,
    feature_positions,
    spatial_bias,
    Wq,
    Wk,
    Wv,
    Wo,
    W_out_w,
    W_out_b,
    W_coord_w,
    W_coord_b,
):
    global _NC_CACHE
    consts = _host_constants(
        np.asarray(Wq, np.float32),
        np.asarray(Wk, np.float32),
        np.asarray(Wv, np.float32),
        np.asarray(Wo, np.float32),
        np.asarray(W_out_w, np.float32),
        np.asarray(W_out_b, np.float32),
        np.asarray(W_coord_w, np.float32),
        np.asarray(W_coord_b, np.float32),
    )
    track_tokens = np.asarray(track_tokens, np.float32)
    feature_map = np.asarray(feature_map, np.float32)
    feature_positions = np.asarray(feature_positions, np.float32)
    spatial_bias = np.asarray(spatial_bias, np.float32)

    in_maps = []
    for c in range(N_CORES):
        t, half = c // 2, c % 2
        qsl = slice(half * QH, (half + 1) * QH)
        m = dict(consts)
        m["tokT"] = _bf(track_tokens[t].T)
        m["posT"] = _f32(feature_positions[t, qsl].T)
        m["biasT"] = _bf(spatial_bias[t][:, qsl])
        m["fmapT"] = _f32(feature_map[t, qsl].T)
        in_maps.append(m)

    if _NC_CACHE is None:
        _NC_CACHE = _build_nc()
    res = run_bass_kernel_spmd(_NC_CACHE, in_maps, core_ids=list(range(N_CORES)))

    out = np.empty((T, HW, DF), np.float32)
    for c in range(N_CORES):
        t, half = c // 2, c % 2
        qsl = slice(half * QH, (half + 1) * QH)
        out[t, qsl, :] = res.results[c]["out"].T
    return out



# revision 13
# speedup vs baseline: 1.3633x; 1.3633x over previous
"""AttentionalSplatting Trainium2 kernel (8 NeuronCores, SPMD).

Sharding: 8 cores = T(4) x HW-halves(2).  Core c handles t = c//2 and pixel
columns [ (c%2)*1152, (c%2+1)*1152 ).  Each core runs the full pipeline for
its (t, pixel-half): coord-proj + 2D RoPE -> Q/K/V proj -> qk-norm ->
scores(+spatial bias) -> softmax -> attend -> Wo -> W_out -> residual.
No cross-core communication is needed; outputs concatenate.

On-chip layout is feature-major ("transposed"): feature/head dims live on
SBUF partitions, pixels/tokens on the free dim.  Scores are computed as
S^T (m on partitions, q free) so the attend matmul consumes exp(S^T)
directly and softmax sums arrive free via a ones-column appended to V.

Attention loop: q is tiled in 3 blocks of 384, heads in two halves of 4.
Each sub-round handles one head-pair x one m-chunk: two identity matmuls
inject the (head-shared) spatial bias into a 2-bank PSUM tile, two
row-tiled (K=32) score matmuls accumulate on top concurrently, one scalar
Exp act (FD=768) produces bf16 E, and the attend matmuls (col-tiled pair)
are emitted with a 1-sub-round lag so the PE never stalls on the scalar
engine.  PSUM budget: score tiles 2 banks x2 bufs + attend accumulators
1 bank x4 = 8 banks.
"""

import math
import sys

import numpy as np

sys.path.insert(0, "/opt/trn_rl_repo")

import ml_dtypes  # noqa: E402

import concourse.bass as bass  # noqa: E402
import concourse.bacc as bacc  # noqa: E402
import concourse.tile as tile  # noqa: E402
from concourse import mybir  # noqa: E402
from concourse.bass_utils import run_bass_kernel_spmd  # noqa: E402

T, M, HW, DF, H = 4, 1024, 2304, 256, 8
DKH = DF // H  # 32
QH = HW // 2  # 1152 pixels per core
SCALE = 1.0 / math.sqrt(DKH)
D_HALF = DF // 2  # 128
D_QUART = DF // 4  # 64
THETA = (100.0 ** (-2.0 * np.arange(D_QUART, dtype=np.float32) / D_HALF)).astype(
    np.float32
)

F32 = mybir.dt.float32
BF16 = mybir.dt.bfloat16
AF = mybir.ActivationFunctionType
BF16NP = ml_dtypes.bfloat16

N_CORES = 8
QB = 384
Q_BLOCKS = [(0, QB), (QB, QB), (2 * QB, QB)]
K_CHUNKS = [(0, 512), (512, 512)]


def _bf(x):
    return np.ascontiguousarray(np.asarray(x, np.float32)).astype(BF16NP)


def _f32(x):
    return np.ascontiguousarray(np.asarray(x, np.float32))


def _host_constants(Wq, Wk, Wv, Wo, W_out_w, W_out_b, W_coord_w, W_coord_b):
    """Shared (core-independent) device constants, host-precomputed."""
    # pair-swapped coord weights for RoPE (swap even/odd output columns)
    perm = np.arange(DF)
    perm = perm.reshape(-1, 2)[:, ::-1].reshape(-1)
    # coord proj with bias folded in via the ones-row of pos_ang
    wc3 = np.concatenate([W_coord_w, W_coord_b[None, :]], axis=0)  # (3, DF)
    wcsw3 = np.concatenate([W_coord_w[:, perm], W_coord_b[perm][None, :]], axis=0)
    # theta lhsT: row0 = signed duplicated theta [-t0,+t0,...], row1 = pi/2
    thp = np.empty((2, D_HALF), np.float32)
    thp[0, 0::2] = -THETA
    thp[0, 1::2] = THETA
    thp[1, :] = math.pi / 2.0
    # block-ones for per-head sum of squares: dtile k maps its 128 feature
    # rows onto head columns 4k..4k+3
    bones = np.zeros((2, 128, 8), np.float32)
    for k in range(2):
        for d in range(128):
            bones[k, d, 4 * k + d // 32] = 1.0
    # expand per-head scalars (8, q) back to the 128 feature rows of dtile k
    exp8 = np.zeros((2, 8, 128), np.float32)
    for k in range(2):
        for d in range(128):
            exp8[k, 4 * k + d // 32, d] = 1.0
    # expand per-head inv-sum (8, q) to paired attend-output rows:
    # pair j holds head 2j at rows 1..33 and head 2j+1 at rows 65..97
    expP = np.zeros((4, 8, 128), np.float32)
    for j in range(4):
        expP[j, 2 * j, 1:33] = 1.0
        expP[j, 2 * j + 1, 65:97] = 1.0
    # Wo rearranged to the paired attend-output row layout (sumexp rows = 0)
    wo_aug = np.zeros((4, 128, DF), np.float32)
    for j in range(4):
        wo_aug[j, 1:33, :] = Wo[(2 * j) * 32 : (2 * j + 1) * 32, :]
        wo_aug[j, 65:97, :] = Wo[(2 * j + 1) * 32 : (2 * j + 2) * 32, :]
    return {
        "wq": _bf(Wq),
        "wk": _bf(Wk),
        "wv": _bf(Wv),
        "wo_aug": _bf(wo_aug),
        "wout": _bf(W_out_w),
        "woutb": _bf(W_out_b).reshape(1, DF),
        "wc3": _f32(wc3),
        "wcsw3": _f32(wcsw3),
        "thp": _f32(thp),
        "bones": _bf(bones),
        "exp8": exp8,
        "expP": expP,
        "ident": _bf(np.eye(128, dtype=np.float32)),
    }


_NC_CACHE = None


def _build_nc():
    nc = bacc.Bacc(
        "TRN2",
        target_bir_lowering=False,
        debug=False,
        enable_asserts=True,
        num_devices=N_CORES,
    )
    d = {}
    inp = lambda name, shape, dt: d.__setitem__(
        name, nc.declare_dram_parameter(name, list(shape), dt, isOutput=False)
    )
    inp("tokT", (DF, M), BF16)
    inp("posA", (3, QH), F32)
    inp("biasT", (M, QH), BF16)
    inp("fmapT", (DF, QH), F32)
    inp("wq", (DF, DF), BF16)
    inp("wk", (DF, DF), BF16)
    inp("wv", (DF, DF), BF16)
    inp("wo_aug", (4, 128, DF), BF16)
    inp("wout", (DF, DF), BF16)
    inp("woutb", (1, DF), BF16)
    inp("wc3", (3, DF), F32)
    inp("wcsw3", (3, DF), F32)
    inp("thp", (2, D_HALF), F32)
    inp("bones", (2, 128, 8), BF16)
    inp("exp8", (2, 8, 128), F32)
    inp("expP", (4, 8, 128), F32)
    inp("ident", (128, 128), BF16)
    out = nc.declare_dram_parameter("out", [DF, QH], F32, isOutput=True)

    import os as _os

    with tile.TileContext(
        nc, trace_sim=bool(_os.environ.get("KERNEL_TRACE_SIM"))
    ) as tc:
        _body(nc, tc, d, out)
    nc.compile()
    return nc


def _body(nc, tc, d, out_dram):
    mm = nc.tensor.matmul
    act = nc.scalar.activation
    dma = nc.sync.dma_start

    with (
        tc.tile_pool(name="const", bufs=1) as cpool,
        tc.tile_pool(name="work", bufs=1) as wpool,
        tc.tile_pool(name="persist", bufs=1) as ppool,
        tc.tile_pool(name="epool", bufs=4) as epool,
        tc.tile_pool(name="psc", bufs=2, space=bass.MemorySpace.PSUM) as psc,
        tc.tile_pool(name="pso", bufs=2, space=bass.MemorySpace.PSUM) as pso,
    ):
        # ---- load constants / inputs to SBUF ----
        # 256-row tensors fold to (128, 2, ...): [:, kt, ...] = rows kt*128..
        def load(name, shape, dt, rearrange=None, **kw):
            t = cpool.tile(list(shape), dt, tag=name)
            src = d[name][:]
            if rearrange is not None:
                src = src.rearrange(rearrange, **kw)
            dma(t[:], src)
            return t

        fold = "(k p) d -> p k d"
        wq = load("wq", (128, 2, DF), BF16, fold, p=128)
        wk = load("wk", (128, 2, DF), BF16, fold, p=128)
        wv = load("wv", (128, 2, DF), BF16, fold, p=128)
        wo_aug = load("wo_aug", (128, 4, DF), BF16, "j p d -> p j d")
        wout = load("wout", (128, 2, DF), BF16, fold, p=128)
        woutb = load("woutb", (1, 2, 128), BF16, "o (k p) -> o k p", k=2)
        wc3 = load("wc3", (3, DF), F32)
        wcsw3 = load("wcsw3", (3, DF), F32)
        thp = load("thp", (2, D_HALF), F32)
        bones = load("bones", (128, 2, 8), BF16, "k p h -> p k h")
        exp8 = load("exp8", (8, 2, 128), F32, "k h d -> h k d")
        expP = load("expP", (8, 4, 128), F32, "j s e -> s j e")
        ident = load("ident", (128, 128), BF16)
        posA = load("posA", (3, QH), F32)
        # rows (y, ones) re-based to partition 0 for the axis-1 angle mms
        posB = cpool.tile([2, QH], F32, tag="posB")
        dma(posB[:], d["posA"][1:3, :])
        # rows (x, ones) adjacent for the axis-0 cos mm
        posC = cpool.tile([2, QH], F32, tag="posC")
        dma(posC[0:1, :], d["posA"][0:1, :])
        dma(posC[1:2, :], d["posA"][2:3, :])
        fmapT = load("fmapT", (128, 2, QH), F32, fold, p=128)
        tokT = load("tokT", (128, 2, M), BF16, fold, p=128)

        bias_sb = []
        for mc in range(8):
            bt = ppool.tile([128, QH], BF16, tag=f"bias{mc}")
            dma(bt[:], d["biasT"][mc * 128 : (mc + 1) * 128, :])
            bias_sb.append(bt)

        ones_q = cpool.tile([1, QB], BF16, tag="ones_q")
        nc.vector.memset(ones_q[:], 1.0)
        lnscale = cpool.tile([128, 1], F32, tag="lnscale")
        nc.vector.memset(lnscale[:], math.log(SCALE))

        # ---- phase A: angle tables, all Sin acts first (one table set) ----
        # sc-tile layout [128, 2, 512]: [:,0,:]=sin(angles), [:,1,:]=cos
        cs = []  # cs[axis] = bf16 [128, 2, QH] (sin, cos)
        for axis in range(2):
            c = ppool.tile([128, 2, QH], BF16, tag=f"cs{axis}")
            cs.append(c)
        for qo, qb in Q_BLOCKS:
            for axis in range(2):
                ps = psc.tile([128, 2, 512], F32, tag="sc")
                src = posA if axis == 0 else posB
                mm(ps[:, 0, 0:qb], thp[0:1, :], src[0:1, qo : qo + qb])
                # cos half: theta*pos + (pi/2)*1 via 2-row contraction
                rhs = (
                    posC[0:2, qo : qo + qb]
                    if axis == 0
                    else posB[0:2, qo : qo + qb]
                )
                mm(ps[:, 1, 0:qb], thp[:, :], rhs)
                act(cs[axis][:, :, qo : qo + qb], ps[:, :, 0:qb], AF.Sin)

        # ---- phase B: coord proj (bias folded) + RoPE (DVE only) ----
        roped = wpool.tile([128, 2, QH], BF16, tag="roped")
        for qo, qb in Q_BLOCKS:
            pq = psc.tile([128, 2, 512], F32, tag="sc")
            pqs = psc.tile([128, 2, 512], F32, tag="sc")
            for dt_i in range(2):
                mm(
                    pq[:, dt_i, 0:qb],
                    wc3[:, dt_i * 128 : (dt_i + 1) * 128],
                    posA[:, qo : qo + qb],
                )
                mm(
                    pqs[:, dt_i, 0:qb],
                    wcsw3[:, dt_i * 128 : (dt_i + 1) * 128],
                    posA[:, qo : qo + qb],
                )
            qin = wpool.tile([128, 2, 512], BF16, tag="qin", bufs=2)
            qins = wpool.tile([128, 2, 512], BF16, tag="qins", bufs=2)
            nc.vector.tensor_copy(qin[:, :, 0:qb], pq[:, :, 0:qb])
            nc.vector.tensor_copy(qins[:, :, 0:qb], pqs[:, :, 0:qb])
            t1 = wpool.tile([128, 2, 512], BF16, tag="t1", bufs=2)
            t2 = wpool.tile([128, 2, 512], BF16, tag="t2", bufs=2)
            for dt_i in range(2):
                # dt0 <- x-axis tables, dt1 <- y-axis
                nc.vector.tensor_mul(
                    t1[:, dt_i, 0:qb],
                    qin[:, dt_i, 0:qb],
                    cs[dt_i][:, 1, qo : qo + qb],
                )
                nc.vector.tensor_mul(
                    t2[:, dt_i, 0:qb],
                    qins[:, dt_i, 0:qb],
                    cs[dt_i][:, 0, qo : qo + qb],
                )
            nc.vector.tensor_add(
                roped[:, :, qo : qo + qb], t1[:, :, 0:qb], t2[:, :, 0:qb]
            )

        # ---- phase C: Q/K projections + qk-norm (Ln/Exp table set) ----
        qnT = ppool.tile([128, 2, QH], BF16, tag="qnT")
        for qo, qb in Q_BLOCKS:
            ps = psc.tile([128, 2, 512], F32, tag="sc")
            for dt_i in range(2):
                for kt in range(2):
                    mm(
                        ps[:, dt_i, 0:qb],
                        wq[:, kt, dt_i * 128 : (dt_i + 1) * 128],
                        roped[:, kt, qo : qo + qb],
                        start=(kt == 0),
                        stop=(kt == 1),
                    )
            tb = wpool.tile([128, 2, 512], BF16, tag="tb", bufs=2)
            nc.vector.tensor_copy(tb[:, :, 0:qb], ps[:, :, 0:qb])
            sq = wpool.tile([128, 2, 512], BF16, tag="sq", bufs=2)
            nc.vector.tensor_mul(sq[:, :, 0:qb], tb[:, :, 0:qb], tb[:, :, 0:qb])
            sq_ps = pso.tile([128, 512], F32, tag="op0")
            for dt_i in range(2):
                mm(
                    sq_ps[0:8, 0:qb],
                    bones[:, dt_i, :],
                    sq[:, dt_i, 0:qb],
                    start=(dt_i == 0),
                    stop=(dt_i == 1),
                )
            lnt = wpool.tile([8, 512], F32, tag="lnt", bufs=2)
            act(lnt[:, 0:qb], sq_ps[0:8, 0:qb], AF.Ln)
            invn = wpool.tile([8, 512], F32, tag="invn", bufs=2)
            act(invn[:, 0:qb], lnt[:, 0:qb], AF.Exp, scale=-0.5, bias=lnscale[0:8, :])
            for dt_i in range(2):
                psx = pso.tile([128, 512], F32, tag="op1")
                mm(psx[:, 0:qb], exp8[:, dt_i, :], invn[:, 0:qb])
                nc.vector.tensor_mul(
                    qnT[:, dt_i, qo : qo + qb], tb[:, dt_i, 0:qb], psx[:, 0:qb]
                )

        knT = ppool.tile([128, 2, M], BF16, tag="knT")
        for ko, kb in K_CHUNKS:
            ps = psc.tile([128, 2, 512], F32, tag="sc")
            for dt_i in range(2):
                for kt in range(2):
                    mm(
                        ps[:, dt_i, 0:kb],
                        wk[:, kt, dt_i * 128 : (dt_i + 1) * 128],
                        tokT[:, kt, ko : ko + kb],
                        start=(kt == 0),
                        stop=(kt == 1),
                    )
            tb = wpool.tile([128, 2, 512], BF16, tag="tb", bufs=2)
            nc.vector.tensor_copy(tb[:, :, 0:kb], ps[:, :, 0:kb])
            sq = wpool.tile([128, 2, 512], BF16, tag="sq", bufs=2)
            nc.vector.tensor_mul(sq[:, :, 0:kb], tb[:, :, 0:kb], tb[:, :, 0:kb])
            sq_ps = pso.tile([128, 512], F32, tag="op0")
            for dt_i in range(2):
                mm(
                    sq_ps[0:8, 0:kb],
                    bones[:, dt_i, :],
                    sq[:, dt_i, 0:kb],
                    start=(dt_i == 0),
                    stop=(dt_i == 1),
                )
            lnt = wpool.tile([8, 512], F32, tag="lnt", bufs=2)
            act(lnt[:, 0:kb], sq_ps[0:8, 0:kb], AF.Ln)
            invn = wpool.tile([8, 512], F32, tag="invn", bufs=2)
            act(invn[:, 0:kb], lnt[:, 0:kb], AF.Exp, scale=-0.5)
            for dt_i in range(2):
                psx = pso.tile([128, 512], F32, tag="op1")
                mm(psx[:, 0:kb], exp8[:, dt_i, :], invn[:, 0:kb])
                nc.vector.tensor_mul(
                    knT[:, dt_i, ko : ko + kb], tb[:, dt_i, 0:kb], psx[:, 0:kb]
                )

        # ---- V (token-major) with ones column:  vsb[mc] = (128, 8, 33) ----
        vsb = []
        for mc in range(8):
            ps = pso.tile([128, 512], F32, tag="op1")
            for kt in range(2):
                mm(
                    ps[:, 0:256],
                    tokT[:, kt, mc * 128 : (mc + 1) * 128],
                    wv[:, kt, :],
                    start=(kt == 0),
                    stop=(kt == 1),
                )
            vt = ppool.tile([128, 8, 33], BF16, tag=f"v{mc}")
            nc.vector.memset(vt[:, :, 0:1], 1.0)
            nc.vector.tensor_copy(
                vt[:, :, 1:33], ps[:, 0:256].rearrange("p (h e) -> p h e", h=8)
            )
            vsb.append(vt)

        # ---- main attention loop ----
        osb = []  # per pair (128, QH) bf16, rows 0/64 = sumexp
        for j in range(4):
            t = ppool.tile([128, QH], BF16, tag=f"osb{j}")
            osb.append(t)

        for qo, qb in Q_BLOCKS:
            for half in range(2):
                o_ps = {}
                for jj in range(2):
                    j = 2 * half + jj
                    o_ps[j] = pso.tile(
                        [128, 512], F32, tag=f"op{jj}", name=f"ops{j}"
                    )
                pend = []

                def emit_attend(item):
                    j, mc, e_t = item
                    h0, h1 = 2 * j, 2 * j + 1
                    mm(
                        o_ps[j][0:33, 0:qb],
                        vsb[mc][:, h0, :],
                        e_t[:, 0, 0:qb],
                        start=(mc == 0),
                        stop=(mc == 7),
                        tile_position=(0, 0),
                    )
                    mm(
                        o_ps[j][64:97, 0:qb],
                        vsb[mc][:, h1, :],
                        e_t[:, 1, 0:qb],
                        start=(mc == 0),
                        stop=(mc == 7),
                        tile_position=(0, 64),
                    )

                for mc in range(8):
                    for jj in range(2):
                        j = 2 * half + jj
                        h0, h1 = 2 * j, 2 * j + 1
                        dt_i = h0 // 4
                        hp0 = (h0 % 4) * 32
                        hp1 = (h1 % 4) * 32
                        X = psc.tile([128, 2, 512], F32, tag="sc")
                        mm(
                            X[:, 0, 0:qb],
                            ident[:],
                            bias_sb[mc][:, qo : qo + qb],
                            start=True,
                            stop=False,
                        )
                        mm(
                            X[:, 1, 0:qb],
                            ident[:],
                            bias_sb[mc][:, qo : qo + qb],
                            start=True,
                            stop=False,
                        )
                        mm(
                            X[:, 0, 0:qb],
                            knT[hp0 : hp0 + 32, dt_i, mc * 128 : (mc + 1) * 128],
                            qnT[hp0 : hp0 + 32, dt_i, qo : qo + qb],
                            start=False,
                            stop=True,
                            tile_position=(hp0, 0),
                        )
                        mm(
                            X[:, 1, 0:qb],
                            knT[hp1 : hp1 + 32, dt_i, mc * 128 : (mc + 1) * 128],
                            qnT[hp1 : hp1 + 32, dt_i, qo : qo + qb],
                            start=False,
                            stop=True,
                            tile_position=(hp1, 0),
                        )
                        e_t = epool.tile([128, 2, 512], BF16, tag="E")
                        act(e_t[:, :, 0:qb], X[:, :, 0:qb], AF.Exp)
                        pend.append((j, mc, e_t))
                        if len(pend) > 1:
                            emit_attend(pend.pop(0))
                while pend:
                    emit_attend(pend.pop(0))
                for jj in range(2):
                    j = 2 * half + jj
                    nc.vector.tensor_copy(
                        osb[j][:, qo : qo + qb], o_ps[j][:, 0:qb]
                    )

        # ---- softmax denominators: gather row 0 of each head, invert ----
        sumE = wpool.tile([8, QH], BF16, tag="sumE")
        for h in range(8):
            j, r = h // 2, 64 * (h % 2)
            dma(sumE[h : h + 1, :], osb[h // 2][r : r + 1, :])
        lnS = wpool.tile([8, QH], F32, tag="lnS")
        act(lnS[:], sumE[:], AF.Ln)
        invS = wpool.tile([8, QH], F32, tag="invS")
        act(invS[:], lnS[:], AF.Exp, scale=-1.0)

        for qo, qb in Q_BLOCKS:
            for j in range(4):
                ps = pso.tile([128, 512], F32, tag=f"op{j % 2}")
                mm(ps[:, 0:qb], expP[:, j, :], invS[:, qo : qo + qb])
                nc.vector.tensor_mul(
                    osb[j][:, qo : qo + qb], osb[j][:, qo : qo + qb], ps[:, 0:qb]
                )

        # ---- output projections + residual (all biases via matmul) ----
        for qo, qb in Q_BLOCKS:
            ps = psc.tile([128, 2, 512], F32, tag="sc")
            for dt_i in range(2):
                for j in range(4):
                    mm(
                        ps[:, dt_i, 0:qb],
                        wo_aug[:, j, dt_i * 128 : (dt_i + 1) * 128],
                        osb[j][:, qo : qo + qb],
                        start=(j == 0),
                        stop=(j == 3),
                    )
            o1b = wpool.tile([128, 2, 512], BF16, tag="o1b", bufs=2)
            nc.vector.tensor_copy(o1b[:, :, 0:qb], ps[:, :, 0:qb])
            ps2 = psc.tile([128, 2, 512], F32, tag="sc")
            for dt_i in range(2):
                for kt in range(2):
                    mm(
                        ps2[:, dt_i, 0:qb],
                        wout[:, kt, dt_i * 128 : (dt_i + 1) * 128],
                        o1b[:, kt, 0:qb],
                        start=(kt == 0),
                        stop=False,
                    )
                mm(
                    ps2[:, dt_i, 0:qb],
                    woutb[:, dt_i, :],
                    ones_q[:, 0:qb],
                    start=False,
                    stop=True,
                )
            res = wpool.tile([128, 2, 512], F32, tag="res", bufs=2)
            nc.vector.tensor_add(
                res[:, :, 0:qb], ps2[:, :, 0:qb], fmapT[:, :, qo : qo + qb]
            )
            for dt_i in range(2):
                dma(
                    out_dram[dt_i * 128 : (dt_i + 1) * 128, qo : qo + qb],
                    res[:, dt_i, 0:qb],
                )


def build_in_maps(inputs):
    consts = _host_constants(
        np.asarray(inputs["Wq"], np.float32),
        np.asarray(inputs["Wk"], np.float32),
        np.asarray(inputs["Wv"], np.float32),
        np.asarray(inputs["Wo"], np.float32),
        np.asarray(inputs["W_out_w"], np.float32),
        np.asarray(inputs["W_out_b"], np.float32),
        np.asarray(inputs["W_coord_w"], np.float32),
        np.asarray(inputs["W_coord_b"], np.float32),
    )
    track_tokens = np.asarray(inputs["track_tokens"], np.float32)
    feature_map = np.asarray(inputs["feature_map"], np.float32)
    feature_positions = np.asarray(inputs["feature_positions"], np.float32)
    spatial_bias = np.asarray(inputs["spatial_bias"], np.float32)

    in_maps = []
    for c in range(N_CORES):
        t, half = c // 2, c % 2
        qsl = slice(half * QH, (half + 1) * QH)
        m = dict(consts)
        m["tokT"] = _bf(track_tokens[t].T)
        pos = feature_positions[t, qsl].T  # (2, QH)
        m["posA"] = _f32(
            np.concatenate([pos, np.ones((1, QH), np.float32)], axis=0)
        )
        m["biasT"] = _bf(spatial_bias[t][:, qsl])
        m["fmapT"] = _f32(feature_map[t, qsl].T)
        in_maps.append(m)
    return in_maps


def kernel(
    track_tokens,
    feature_map,
    feature_positions,
    spatial_bias,
    Wq,
    Wk,
    Wv,
    Wo,
    W_out_w,
    W_out_b,
    W_coord_w,
    W_coord_b,
):
    global _NC_CACHE
    in_maps = build_in_maps(
        dict(
            track_tokens=track_tokens,
            feature_map=feature_map,
            feature_positions=feature_positions,
            spatial_bias=spatial_bias,
            Wq=Wq,
            Wk=Wk,
            Wv=Wv,
            Wo=Wo,
            W_out_w=W_out_w,
            W_out_b=W_out_b,
            W_coord_w=W_coord_w,
            W_coord_b=W_coord_b,
        )
    )

    if _NC_CACHE is None:
        _NC_CACHE = _build_nc()
    res = run_bass_kernel_spmd(_NC_CACHE, in_maps, core_ids=list(range(N_CORES)))

    out = np.empty((T, HW, DF), np.float32)
    for c in range(N_CORES):
        t, half = c // 2, c % 2
        qsl = slice(half * QH, (half + 1) * QH)
        out[t, qsl, :] = res.results[c]["out"].T
    return out


# revision 20
# speedup vs baseline: 1.4394x; 1.0558x over previous
"""AttentionalSplatting Trainium2 kernel (8 NeuronCores, SPMD).

Sharding: 8 cores = T(4) x HW-halves(2).  Core c handles t = c//2 and pixel
columns [ (c%2)*1152, (c%2+1)*1152 ).  Each core runs the full pipeline for
its (t, pixel-half): coord-proj + 2D RoPE -> Q/K/V proj -> qk-norm ->
scores(+spatial bias) -> softmax -> attend -> Wo -> W_out -> residual.
No cross-core communication is needed; outputs concatenate.

On-chip layout is feature-major ("transposed"): feature/head dims live on
SBUF partitions, pixels/tokens on the free dim.  Scores are computed as
S^T (m on partitions, q free) so the attend matmul consumes exp(S^T)
directly and softmax sums arrive free via a ones-column appended to V.

Attention loop: q is tiled in 3 blocks of 384, heads in two halves of 4.
Each sub-round handles one head-pair x one m-chunk: two identity matmuls
inject the (head-shared) spatial bias into a 2-bank PSUM tile, two
row-tiled (K=32) score matmuls accumulate on top concurrently, one scalar
Exp act (FD=768) produces bf16 E, and the attend matmuls (col-tiled pair)
are emitted with a 1-sub-round lag so the PE never stalls on the scalar
engine.  PSUM budget: score tiles 2 banks x2 bufs + attend accumulators
1 bank x4 = 8 banks.
"""

import math
import sys

import numpy as np

sys.path.insert(0, "/opt/trn_rl_repo")

import ml_dtypes  # noqa: E402

import concourse.bass as bass  # noqa: E402
import concourse.bacc as bacc  # noqa: E402
import concourse.tile as tile  # noqa: E402
from concourse import mybir  # noqa: E402
from concourse.bass_utils import run_bass_kernel_spmd  # noqa: E402

T, M, HW, DF, H = 4, 1024, 2304, 256, 8
DKH = DF // H  # 32
QH = HW // 2  # 1152 pixels per core
SCALE = 1.0 / math.sqrt(DKH)
D_HALF = DF // 2  # 128
D_QUART = DF // 4  # 64
THETA = (100.0 ** (-2.0 * np.arange(D_QUART, dtype=np.float32) / D_HALF)).astype(
    np.float32
)

F32 = mybir.dt.float32
BF16 = mybir.dt.bfloat16
AF = mybir.ActivationFunctionType
BF16NP = ml_dtypes.bfloat16

N_CORES = 8
QB = 384
Q_BLOCKS = [(0, QB), (QB, QB), (2 * QB, QB)]
K_CHUNKS = [(0, 512), (512, 512)]


def _bf(x):
    return np.ascontiguousarray(np.asarray(x, np.float32)).astype(BF16NP)


def _f32(x):
    return np.ascontiguousarray(np.asarray(x, np.float32))


def _host_constants(Wq, Wk, Wv, Wo, W_out_w, W_out_b, W_coord_w, W_coord_b):
    """Shared (core-independent) device constants, host-precomputed."""
    # pair-swapped coord weights for RoPE (swap even/odd output columns)
    perm = np.arange(DF)
    perm = perm.reshape(-1, 2)[:, ::-1].reshape(-1)
    # coord proj with bias folded in via the ones-row of pos_ang
    wc3 = np.concatenate([W_coord_w, W_coord_b[None, :]], axis=0)  # (3, DF)
    wcsw3 = np.concatenate([W_coord_w[:, perm], W_coord_b[perm][None, :]], axis=0)
    # theta lhsT: row0 = signed duplicated theta [-t0,+t0,...], row1 = pi/2
    thp = np.empty((2, D_HALF), np.float32)
    thp[0, 0::2] = -THETA
    thp[0, 1::2] = THETA
    thp[1, :] = math.pi / 2.0
    # block-ones for per-head sum of squares: dtile k maps its 128 feature
    # rows onto head columns 4k..4k+3
    bones = np.zeros((2, 128, 8), np.float32)
    for k in range(2):
        for d in range(128):
            bones[k, d, 4 * k + d // 32] = 1.0
    # expand per-head scalars (8, q) back to the 128 feature rows of dtile k
    exp8 = np.zeros((2, 8, 128), np.float32)
    for k in range(2):
        for d in range(128):
            exp8[k, 4 * k + d // 32, d] = 1.0
    # expand per-head inv-sum (8, q) to paired attend-output rows:
    # pair j holds head 2j at rows 1..33 and head 2j+1 at rows 65..97
    expP = np.zeros((4, 8, 128), np.float32)
    for j in range(4):
        expP[j, 2 * j, 1:33] = 1.0
        expP[j, 2 * j + 1, 65:97] = 1.0
    # Wo rearranged to the paired attend-output row layout (sumexp rows = 0)
    wo_aug = np.zeros((4, 128, DF), np.float32)
    for j in range(4):
        wo_aug[j, 1:33, :] = Wo[(2 * j) * 32 : (2 * j + 1) * 32, :]
        wo_aug[j, 65:97, :] = Wo[(2 * j + 1) * 32 : (2 * j + 2) * 32, :]
    return {
        "wq": _bf(Wq),
        "wk": _bf(Wk),
        "wv": _bf(Wv),
        "wo_aug": _bf(wo_aug),
        "wout": _bf(W_out_w),
        "woutb": _bf(W_out_b).reshape(1, DF),
        "wc3": _f32(wc3),
        "wcsw3": _f32(wcsw3),
        "thp": _f32(thp),
        "bones": _bf(bones),
        "exp8": exp8,
        "expP": expP,
        "ident": _bf(np.eye(128, dtype=np.float32)),
    }


_NC_CACHE = None


def _build_nc():
    nc = bacc.Bacc(
        "TRN2",
        target_bir_lowering=False,
        debug=False,
        enable_asserts=True,
        num_devices=N_CORES,
    )
    d = {}
    inp = lambda name, shape, dt: d.__setitem__(
        name, nc.declare_dram_parameter(name, list(shape), dt, isOutput=False)
    )
    inp("tokT", (DF, M), BF16)
    inp("posA", (3, QH), F32)
    inp("biasT", (M, QH), BF16)
    inp("fmapT", (DF, QH), F32)
    inp("wq", (DF, DF), BF16)
    inp("wk", (DF, DF), BF16)
    inp("wv", (DF, DF), BF16)
    inp("wo_aug", (4, 128, DF), BF16)
    inp("wout", (DF, DF), BF16)
    inp("woutb", (1, DF), BF16)
    inp("wc3", (3, DF), F32)
    inp("wcsw3", (3, DF), F32)
    inp("thp", (2, D_HALF), F32)
    inp("bones", (2, 128, 8), BF16)
    inp("exp8", (2, 8, 128), F32)
    inp("expP", (4, 8, 128), F32)
    inp("ident", (128, 128), BF16)
    out = nc.declare_dram_parameter("out", [DF, QH], F32, isOutput=True)

    import os as _os

    with tile.TileContext(
        nc, trace_sim=bool(_os.environ.get("KERNEL_TRACE_SIM"))
    ) as tc:
        _body(nc, tc, d, out)
    nc.compile()
    return nc


def _body(nc, tc, d, out_dram):
    mm = nc.tensor.matmul
    act = nc.scalar.activation
    dma = nc.sync.dma_start

    with (
        tc.tile_pool(name="const", bufs=1) as cpool,
        tc.tile_pool(name="work", bufs=1) as wpool,
        tc.tile_pool(name="persist", bufs=1) as ppool,
        tc.tile_pool(name="epool", bufs=6) as epool,
        tc.tile_pool(name="psc", bufs=2, space=bass.MemorySpace.PSUM) as psc,
        tc.tile_pool(name="pso", bufs=2, space=bass.MemorySpace.PSUM) as pso,
    ):
        # ---- load constants / inputs to SBUF ----
        # 256-row tensors fold to (128, 2, ...): [:, kt, ...] = rows kt*128..
        def load(name, shape, dt, rearrange=None, **kw):
            t = cpool.tile(list(shape), dt, tag=name)
            src = d[name][:]
            if rearrange is not None:
                src = src.rearrange(rearrange, **kw)
            dma(t[:], src)
            return t

        fold = "(k p) d -> p k d"
        # critical-path loads first (pre-phase + attention start)
        posA = load("posA", (3, QH), F32)
        # rows (y, ones) re-based to partition 0 for the axis-1 angle mms
        posB = cpool.tile([2, QH], F32, tag="posB")
        dma(posB[:], d["posA"][1:3, :])
        # rows (x, ones) adjacent for the axis-0 cos mm
        posC = cpool.tile([2, QH], F32, tag="posC")
        dma(posC[0:1, :], d["posA"][0:1, :])
        dma(posC[1:2, :], d["posA"][2:3, :])
        thp = load("thp", (2, D_HALF), F32)
        wc3 = load("wc3", (3, DF), F32)
        wcsw3 = load("wcsw3", (3, DF), F32)
        ident = load("ident", (128, 128), BF16)
        bones = load("bones", (128, 2, 8), BF16, "k p h -> p k h")
        exp8 = load("exp8", (8, 2, 128), F32, "k h d -> h k d")
        tokT = load("tokT", (128, 2, M), BF16, fold, p=128)
        wk = load("wk", (128, 2, DF), BF16, fold, p=128)
        wv = load("wv", (128, 2, DF), BF16, fold, p=128)
        wq = load("wq", (128, 2, DF), BF16, fold, p=128)

        bias_sb = []
        for mc in range(8):
            bt = ppool.tile([128, QH], BF16, tag=f"bias{mc}")
            dma(bt[:], d["biasT"][mc * 128 : (mc + 1) * 128, :])
            bias_sb.append(bt)

        # tail-phase loads (emitted late in the DMA queue on purpose)
        wo_aug = load("wo_aug", (128, 4, DF), BF16, "j p d -> p j d")
        wout = load("wout", (128, 2, DF), BF16, fold, p=128)
        woutb = load("woutb", (1, 2, 128), BF16, "o (k p) -> o k p", k=2)
        expP = load("expP", (8, 4, 128), F32, "j s e -> s j e")
        fmapT = load("fmapT", (128, 2, QH), F32, fold, p=128)

        ones_q = cpool.tile([1, QB], BF16, tag="ones_q")
        nc.vector.memset(ones_q[:], 1.0)

        # ---- phase A: angle tables, all Sin acts first (one table set) ----
        # sc-tile layout [128, 2, 512]: [:,0,:]=sin(angles), [:,1,:]=cos
        cs = []  # cs[axis] = bf16 [128, 2, QH] (sin, cos)
        for axis in range(2):
            c = ppool.tile([128, 2, QH], BF16, tag=f"cs{axis}")
            cs.append(c)
        for qo, qb in Q_BLOCKS:
            for axis in range(2):
                ps = psc.tile([128, 2, 512], F32, tag="sc")
                src = posA if axis == 0 else posB
                mm(ps[:, 0, 0:qb], thp[0:1, :], src[0:1, qo : qo + qb])
                # cos half: theta*pos + (pi/2)*1 via 2-row contraction
                rhs = (
                    posC[0:2, qo : qo + qb]
                    if axis == 0
                    else posB[0:2, qo : qo + qb]
                )
                mm(ps[:, 1, 0:qb], thp[:, :], rhs)
                act(cs[axis][:, :, qo : qo + qb], ps[:, :, 0:qb], AF.Sin)

        knT = ppool.tile([128, 2, M], BF16, tag="knT")
        qnT = ppool.tile([128, 2, QH], BF16, tag="qnT")
        lnscale = cpool.tile([128, 1], F32, tag="lnscale")
        nc.vector.memset(lnscale[:], math.log(SCALE))
        zero_c = cpool.tile([128, 1], F32, tag="zero_c")
        nc.vector.memset(zero_c[:], 0.0)

        # qk-norm runs in two passes so the scalar engine sees all Ln acts
        # then all Exp acts (one table load each).
        segs = []  # (tb, lnt, out_tile, off, n, ln_bias)

        def norm_pass1(ps, n, idx):
            tb = wpool.tile([128, 2, 512], BF16, tag=f"tb{idx}")
            nc.vector.tensor_copy(tb[:, :, 0:n], ps[:, :, 0:n])
            sq = wpool.tile([128, 2, 512], BF16, tag="sq", bufs=2)
            nc.vector.tensor_mul(sq[:, :, 0:n], tb[:, :, 0:n], tb[:, :, 0:n])
            sq_ps = pso.tile([128, 512], F32, tag="op0")
            for dt_i in range(2):
                mm(
                    sq_ps[0:8, 0:n],
                    bones[:, dt_i, :],
                    sq[:, dt_i, 0:n],
                    start=(dt_i == 0),
                    stop=(dt_i == 1),
                )
            lnt = wpool.tile([8, 512], F32, tag=f"lnt{idx}")
            act(lnt[:, 0:n], sq_ps[0:8, 0:n], AF.Ln)
            return tb, lnt

        # ---- K projection + norm pass 1 ----
        for ci, (ko, kb) in enumerate(K_CHUNKS):
            ps = psc.tile([128, 2, 512], F32, tag="sc")
            for dt_i in range(2):
                for kt in range(2):
                    mm(
                        ps[:, dt_i, 0:kb],
                        wk[:, kt, dt_i * 128 : (dt_i + 1) * 128],
                        tokT[:, kt, ko : ko + kb],
                        start=(kt == 0),
                        stop=(kt == 1),
                    )
            tb, lnt = norm_pass1(ps, kb, f"k{ci}")
            segs.append((tb, lnt, knT, ko, kb, zero_c))

        # ---- phase B: coord proj (bias folded) + RoPE (DVE only) ----
        roped = wpool.tile([128, 2, QH], BF16, tag="roped")
        for qo, qb in Q_BLOCKS:
            pq = psc.tile([128, 2, 512], F32, tag="sc")
            pqs = psc.tile([128, 2, 512], F32, tag="sc")
            for dt_i in range(2):
                mm(
                    pq[:, dt_i, 0:qb],
                    wc3[:, dt_i * 128 : (dt_i + 1) * 128],
                    posA[:, qo : qo + qb],
                )
                mm(
                    pqs[:, dt_i, 0:qb],
                    wcsw3[:, dt_i * 128 : (dt_i + 1) * 128],
                    posA[:, qo : qo + qb],
                )
            qin = wpool.tile([128, 2, 512], BF16, tag="qin", bufs=2)
            qins = wpool.tile([128, 2, 512], BF16, tag="qins", bufs=2)
            nc.vector.tensor_copy(qin[:, :, 0:qb], pq[:, :, 0:qb])
            nc.vector.tensor_copy(qins[:, :, 0:qb], pqs[:, :, 0:qb])
            t1 = wpool.tile([128, 2, 512], BF16, tag="t1", bufs=2)
            t2 = wpool.tile([128, 2, 512], BF16, tag="t2", bufs=2)
            for dt_i in range(2):
                # dt0 <- x-axis tables, dt1 <- y-axis
                nc.vector.tensor_mul(
                    t1[:, dt_i, 0:qb],
                    qin[:, dt_i, 0:qb],
                    cs[dt_i][:, 1, qo : qo + qb],
                )
                nc.vector.tensor_mul(
                    t2[:, dt_i, 0:qb],
                    qins[:, dt_i, 0:qb],
                    cs[dt_i][:, 0, qo : qo + qb],
                )
            nc.vector.tensor_add(
                roped[:, :, qo : qo + qb], t1[:, :, 0:qb], t2[:, :, 0:qb]
            )

        # ---- Q projection + norm pass 1 ----
        for qi, (qo, qb) in enumerate(Q_BLOCKS):
            ps = psc.tile([128, 2, 512], F32, tag="sc")
            for dt_i in range(2):
                for kt in range(2):
                    mm(
                        ps[:, dt_i, 0:qb],
                        wq[:, kt, dt_i * 128 : (dt_i + 1) * 128],
                        roped[:, kt, qo : qo + qb],
                        start=(kt == 0),
                        stop=(kt == 1),
                    )
            tb, lnt = norm_pass1(ps, qb, f"q{qi}")
            segs.append((tb, lnt, qnT, qo, qb, lnscale))

        # ---- norm pass 2: all Exp acts, then expand + scale ----
        for si, (tb, lnt, out_t, off, n, ln_bias) in enumerate(segs):
            invn = wpool.tile([8, 512], F32, tag="invn", bufs=3)
            act(
                invn[:, 0:n], lnt[:, 0:n], AF.Exp, scale=-0.5, bias=ln_bias[0:8, :]
            )
            for dt_i in range(2):
                psx = pso.tile([128, 512], F32, tag="op1")
                mm(psx[:, 0:n], exp8[:, dt_i, :], invn[:, 0:n])
                nc.vector.tensor_mul(
                    out_t[:, dt_i, off : off + n], tb[:, dt_i, 0:n], psx[:, 0:n]
                )

        # ---- V (token-major) with ones column:  vsb[mc] = (128, 8, 33) ----
        vsb = []
        for mc in range(8):
            ps = pso.tile([128, 512], F32, tag="op1")
            for kt in range(2):
                mm(
                    ps[:, 0:256],
                    tokT[:, kt, mc * 128 : (mc + 1) * 128],
                    wv[:, kt, :],
                    start=(kt == 0),
                    stop=(kt == 1),
                )
            vt = ppool.tile([128, 8, 33], BF16, tag=f"v{mc}")
            nc.vector.memset(vt[:, :, 0:1], 1.0)
            nc.vector.tensor_copy(
                vt[:, :, 1:33], ps[:, 0:256].rearrange("p (h e) -> p h e", h=8)
            )
            vsb.append(vt)

        # ---- main attention loop ----
        osb = []  # per pair (128, QH) bf16, rows 0/64 = sumexp
        for j in range(4):
            t = ppool.tile([128, QH], BF16, tag=f"osb{j}")
            osb.append(t)

        for qo, qb in Q_BLOCKS:
            for half in range(2):
                dt_i = half
                o_ps = {}
                for jj in range(2):
                    j = 2 * half + jj
                    o_ps[j] = pso.tile(
                        [128, 512], F32, tag=f"op{jj}", name=f"ops{j}"
                    )
                pend = []

                def emit_attend(item):
                    j, mc, e_t = item
                    h0, h1 = 2 * j, 2 * j + 1
                    mm(
                        o_ps[j][0:33, 0:qb],
                        vsb[mc][:, h0, :],
                        e_t[:, 0, 0:qb],
                        start=(mc == 0),
                        stop=(mc == 7),
                        tile_position=(0, 0),
                    )
                    mm(
                        o_ps[j][64:97, 0:qb],
                        vsb[mc][:, h1, :],
                        e_t[:, 1, 0:qb],
                        start=(mc == 0),
                        stop=(mc == 7),
                        tile_position=(0, 64),
                    )

                for mc in range(8):
                    # one cluster: both head-pairs of this half x one m-chunk
                    Xs = []
                    for jj in range(2):
                        X = psc.tile([128, 2, 512], F32, tag="sc", name=f"x{jj}")
                        Xs.append(X)
                    # batched bias injection: 4 mms sharing the ident weights
                    for jj in range(2):
                        for hh in range(2):
                            mm(
                                Xs[jj][:, hh, 0:qb],
                                ident[:],
                                bias_sb[mc][:, qo : qo + qb],
                                start=True,
                                stop=False,
                            )
                    # batched scores: 4 row-tiled mms run concurrently
                    for jj in range(2):
                        j = 2 * half + jj
                        for hh in range(2):
                            h = 2 * j + hh
                            hp = (h % 4) * 32
                            mm(
                                Xs[jj][:, hh, 0:qb],
                                knT[hp : hp + 32, dt_i, mc * 128 : (mc + 1) * 128],
                                qnT[hp : hp + 32, dt_i, qo : qo + qb],
                                start=False,
                                stop=True,
                                tile_position=(hp, 0),
                            )
                    for jj in range(2):
                        j = 2 * half + jj
                        e_t = epool.tile([128, 2, 512], BF16, tag="E")
                        act(e_t[:, :, 0:qb], Xs[jj][:, :, 0:qb], AF.Exp)
                        pend.append((j, mc, e_t))
                    if len(pend) > 2:
                        emit_attend(pend.pop(0))
                        emit_attend(pend.pop(0))
                while pend:
                    emit_attend(pend.pop(0))
                for jj in range(2):
                    j = 2 * half + jj
                    nc.vector.tensor_copy(
                        osb[j][:, qo : qo + qb], o_ps[j][:, 0:qb]
                    )

        # ---- softmax denominators: gather row 0 of each head, invert ----
        sumE = wpool.tile([8, QH], BF16, tag="sumE")
        for h in range(8):
            j, r = h // 2, 64 * (h % 2)
            dma(sumE[h : h + 1, :], osb[h // 2][r : r + 1, :])
        sumEf = wpool.tile([8, QH], F32, tag="sumEf")
        nc.vector.tensor_copy(sumEf[:], sumE[:])
        invS = wpool.tile([8, QH], F32, tag="invS")
        nc.vector.reciprocal(invS[:], sumEf[:])

        for qo, qb in Q_BLOCKS:
            for j in range(4):
                ps = pso.tile([128, 512], F32, tag=f"op{j % 2}")
                mm(ps[:, 0:qb], expP[:, j, :], invS[:, qo : qo + qb])
                nc.vector.tensor_mul(
                    osb[j][:, qo : qo + qb], osb[j][:, qo : qo + qb], ps[:, 0:qb]
                )

        # ---- output projections + residual (all biases via matmul) ----
        for qo, qb in Q_BLOCKS:
            ps = psc.tile([128, 2, 512], F32, tag="sc")
            for dt_i in range(2):
                for j in range(4):
                    mm(
                        ps[:, dt_i, 0:qb],
                        wo_aug[:, j, dt_i * 128 : (dt_i + 1) * 128],
                        osb[j][:, qo : qo + qb],
                        start=(j == 0),
                        stop=(j == 3),
                    )
            o1b = wpool.tile([128, 2, 512], BF16, tag="o1b", bufs=2)
            nc.vector.tensor_copy(o1b[:, :, 0:qb], ps[:, :, 0:qb])
            ps2 = psc.tile([128, 2, 512], F32, tag="sc")
            for dt_i in range(2):
                for kt in range(2):
                    mm(
                        ps2[:, dt_i, 0:qb],
                        wout[:, kt, dt_i * 128 : (dt_i + 1) * 128],
                        o1b[:, kt, 0:qb],
                        start=(kt == 0),
                        stop=False,
                    )
                mm(
                    ps2[:, dt_i, 0:qb],
                    woutb[:, dt_i, :],
                    ones_q[:, 0:qb],
                    start=False,
                    stop=True,
                )
            res = wpool.tile([128, 2, 512], F32, tag="res", bufs=2)
            nc.vector.tensor_add(
                res[:, :, 0:qb], ps2[:, :, 0:qb], fmapT[:, :, qo : qo + qb]
            )
            for dt_i in range(2):
                dma(
                    out_dram[dt_i * 128 : (dt_i + 1) * 128, qo : qo + qb],
                    res[:, dt_i, 0:qb],
                )


def build_in_maps(inputs):
    consts = _host_constants(
        np.asarray(inputs["Wq"], np.float32),
        np.asarray(inputs["Wk"], np.float32),
        np.asarray(inputs["Wv"], np.float32),
        np.asarray(inputs["Wo"], np.float32),
        np.asarray(inputs["W_out_w"], np.float32),
        np.asarray(inputs["W_out_b"], np.float32),
        np.asarray(inputs["W_coord_w"], np.float32),
        np.asarray(inputs["W_coord_b"], np.float32),
    )
    track_tokens = np.asarray(inputs["track_tokens"], np.float32)
    feature_map = np.asarray(inputs["feature_map"], np.float32)
    feature_positions = np.asarray(inputs["feature_positions"], np.float32)
    spatial_bias = np.asarray(inputs["spatial_bias"], np.float32)

    in_maps = []
    for c in range(N_CORES):
        t, half = c // 2, c % 2
        qsl = slice(half * QH, (half + 1) * QH)
        m = dict(consts)
        m["tokT"] = _bf(track_tokens[t].T)
        pos = feature_positions[t, qsl].T  # (2, QH)
        m["posA"] = _f32(
            np.concatenate([pos, np.ones((1, QH), np.float32)], axis=0)
        )
        m["biasT"] = _bf(spatial_bias[t][:, qsl])
        m["fmapT"] = _f32(feature_map[t, qsl].T)
        in_maps.append(m)
    return in_maps


def kernel(
    track_tokens,
    feature_map,
    feature_positions,
    spatial_bias,
    Wq,
    Wk,
    Wv,
    Wo,
    W_out_w,
    W_out_b,
    W_coord_w,
    W_coord_b,
):
    global _NC_CACHE
    in_maps = build_in_maps(
        dict(
            track_tokens=track_tokens,
            feature_map=feature_map,
            feature_positions=feature_positions,
            spatial_bias=spatial_bias,
            Wq=Wq,
            Wk=Wk,
            Wv=Wv,
            Wo=Wo,
            W_out_w=W_out_w,
            W_out_b=W_out_b,
            W_coord_w=W_coord_w,
            W_coord_b=W_coord_b,
        )
    )

    if _NC_CACHE is None:
        _NC_CACHE = _build_nc()
    res = run_bass_kernel_spmd(_NC_CACHE, in_maps, core_ids=list(range(N_CORES)))

    out = np.empty((T, HW, DF), np.float32)
    for c in range(N_CORES):
        t, half = c // 2, c % 2
        qsl = slice(half * QH, (half + 1) * QH)
        out[t, qsl, :] = res.results[c]["out"].T
    return out
